# revision 30
# baseline (speedup 1.0000x reference)
"""Trainium2 Bass kernel for nn_Energy_layer (stencil energy/flux losses).

Contract: kernel(layout, heat, flow) takes FULL inputs, returns the FULL
output tuple (mse_energy + mse_flux, heat_bc, eq_mask) matching reference().

Strategy: rows are sharded across 8 NeuronCores (256 rows each, 1-row halo
supplied host-side via overlapping slices of the reflect-padded arrays).
On each core the image is processed as 512 patches of 8x256 interior with a
1-ring halo; each SBUF partition holds one patch, so every 3x3-stencil shift
is a free-dimension shift. Masking/loss algebra uses the identity

    loss_energy = adv*em + D,  loss_flux = |D| - F_amt(b),
    D = fdx*[b in {4,8,11}] + fdy*[b in {5,8,9}] + bdx*[b in {6,9,10}]
        + bdy*[b in {7,10,11}]

which collapses the 10 reference masks into 4 mask-sums shared by both
losses. Per-partition partial sums of loss^2 are accumulated on the scalar
engine (Square activation with accum_out) and reduced on host.
"""

import functools

import numpy as np

import concourse.bass as bass
import concourse.mybir as mybir
import concourse.tile as tile
from concourse.ap import AP
from concourse.vector_clock import ScopedClock

A = mybir.AluOpType
F32 = mybir.dt.float32
BF16 = mybir.dt.bfloat16
U8 = mybir.dt.uint8
ACTF = mybir.ActivationFunctionType

# ---------------------------------------------------------------- geometry
H, W = 2048, 4096
NCORES = 8
R = H // NCORES            # 256 rows per core
G = 8                      # patch interior rows
WCH = 256                  # patch interior cols
CCH = W // WCH             # 16 col chunks
BPB = 8                    # bands per block (8 bands x 16 chunks = 128 patches)
NBLK = (R // G) // BPB     # 4 blocks per core
WIN = W + 2                # 4098 input cols (with reflect halo)
RIN = R + 2                # 258 input rows (with halo)

FLUX = np.float32(300.0 * (6.0 / 4096))
SQRT2 = np.float32(1.41421)
SQ2F = np.float32(SQRT2 * FLUX)
DELTA = np.float32(SQ2F - FLUX)   # exact: F + DELTA == fl(SQRT2*F)

_WAIT_CAP = 1


# ------------------------------------------------- walrus compat workarounds
def _patched_drain_and_barrier(self, tick_clock, wait_clock):
    # This walrus build rejects instructions carrying more than one sync
    # wait; split the tile tail-drain's waits across multiple drains.
    nc = self.nc
    drain_inst = nc.sync.drain()
    wait_clock.add_sem_waits(
        drain_inst.ins, ScopedClock({None: tick_clock.global_clock})
    )
    si = drain_inst.ins.sync_info
    waits = list(si.on_wait or [])
    if len(waits) > _WAIT_CAP:
        si.on_wait = waits[:_WAIT_CAP]
        rest = waits[_WAIT_CAP:]
        while rest:
            d2 = nc.sync.drain()
            si2 = d2.ins.sync_info
            if si2 is None:
                d2.ins.sync_info = mybir.SyncInfo(
                    on_update=[], on_wait=rest[:_WAIT_CAP]
                )
            else:
                si2.on_wait = rest[:_WAIT_CAP]
            rest = rest[_WAIT_CAP:]
    nc.all_engine_barrier()
    assert self.sems is not None
    popped = nc._tile_sem_poison_stack.pop()
    assert popped is self._sem_poison
    nc.clear_and_free_semaphores(list(self.sems.allocated().values()))
    nc.all_engine_barrier()


tile.TileContext._drain_and_barrier = _patched_drain_and_barrier


def _split_excess_waits(nc, cap=_WAIT_CAP):
    # Move excess sem waits onto NoOps inserted before the instruction on
    # the same engine (program order preserves the wait semantics).
    for fn in nc.m.functions:
        for blk in fn.blocks:
            out = []
            changed = False
            for inst in blk.instructions:
                si = inst.sync_info
                waits = list(si.on_wait) if si is not None and si.on_wait else []
                if len(waits) > cap:
                    excess = waits[:-cap]
                    si.on_wait = waits[-cap:]
                    for j in range(0, len(excess), cap):
                        nop = mybir.InstNoOp(
                            name=f"{inst.name}-wsplit{j}", ins=[], outs=[]
                        )
                        nop.engine = inst.engine
                        nop.sync_info = mybir.SyncInfo(
                            on_update=[], on_wait=excess[j : j + cap]
                        )
                        out.append(nop)
                    changed = True
                out.append(inst)
            if changed:
                blk.instructions = out


# ------------------------------------------------------------ kernel build
def _emit_block(nc, pio, pscr, pacc, handles, blk):
    heat_h, b_h, u_h, v_h, hbc_h, eq_h, acc_h = handles

    ht = pio.tile([128, G + 2, WCH + 2], F32, tag="heat")
    bt = pio.tile([128, G + 2, WCH + 2], F32, tag="b")
    bbf = pio.tile([128, G, WCH], BF16, tag="bbf")
    ut = pio.tile([128, G, WCH], BF16, tag="u")
    vt = pio.tile([128, G, WCH], BF16, tag="v")
    hbc = pio.tile([128, G + 2, WCH + 2], F32, tag="hbc")
    eqt = pio.tile([128, G, WCH], BF16, tag="eq")

    # bf16 copies of hbc; b0 plain (E/W views 4B-aligned), b1 shifted left
    # by one col (C/N/S views 4B-aligned) so DVE 2x mode engages everywhere
    hb0 = pio.tile([128, G + 2, WCH + 2], BF16, tag="hb0")
    hb1 = pio.tile([128, G + 2, WCH + 2], BF16, tag="hb1")

    fdx = pscr.tile([128, G, WCH], BF16, tag="fdx")
    fdy = pscr.tile([128, G, WCH], BF16, tag="fdy")
    bdx = pscr.tile([128, G, WCH], BF16, tag="bdx")
    bdy = pscr.tile([128, G, WCH], BF16, tag="bdy")
    dxh = pscr.tile([128, G, WCH], BF16, tag="dxh")
    syh = pscr.tile([128, G, WCH], BF16, tag="syh")
    em2 = pscr.tile([128, G, WCH], BF16, tag="em2")

    acc_e = pacc.tile([128, 1], F32, tag=f"acce{blk}")
    acc_f = pacc.tile([128, 1], F32, tag=f"accf{blk}")

    sl = slice(blk * 128, (blk + 1) * 128)
    nc.sync.dma_start(out=ht[:], in_=heat_h.ap()[sl, :])
    nc.gpsimd.dma_start(out=bt[:], in_=b_h.ap()[sl, :])  # u8 -> f32 cast
    # u8 -> bf16 cast load of the interior of b (strided in DRAM)
    b_int_src = AP(
        b_h,
        blk * 128 * FHALO + (WCH + 2) + 1,
        [[FHALO, 128], [WCH + 2, G], [1, WCH]],
    )
    nc.gpsimd.dma_start(out=bbf[:], in_=b_int_src)
    nc.sync.dma_start(out=ut[:], in_=u_h.ap()[sl, :])
    nc.sync.dma_start(out=vt[:], in_=v_h.ap()[sl, :])

    import os

    gp_ops = set(os.environ.get("GP_OPS", "diffs,prods").split(","))
    gp = nc.gpsimd
    v = nc.vector
    sc = nc.scalar
    e_diff = gp if "diffs" in gp_ops else v
    e_prod = gp if "prods" in gp_ops else v
    e_dadd = gp if "dadds" in gp_ops else v
    e_adv = gp if "adv" in gp_ops else v

    # hbc = heat * (b != 1)  (full halo tile, f32 - this is an output)
    v.scalar_tensor_tensor(hbc[:], bt[:], 1.0, ht[:], A.not_equal, A.mult)
    # bf16 copies for the stencil math
    sc.copy(hb0[:], hbc[:])
    sc.copy(hb1[:, :, 0 : WCH + 1], hbc[:, :, 1 : WCH + 2])

    # interior views (C at halo col j; even element offsets everywhere)
    C = hb1[:, 1 : G + 1, 0:WCH]
    E = hb0[:, 1 : G + 1, 2 : WCH + 2]
    Wv = hb0[:, 1 : G + 1, 0:WCH]
    Nv = hb1[:, 0:G, 0:WCH]
    Sv = hb1[:, 2 : G + 2, 0:WCH]
    bi = bbf[:]

    e_diff.tensor_tensor(fdx[:], E, C, A.subtract)
    e_diff.tensor_tensor(fdy[:], Nv, C, A.subtract)
    e_diff.tensor_tensor(bdx[:], C, Wv, A.subtract)
    e_diff.tensor_tensor(bdy[:], C, Sv, A.subtract)
    e_diff.tensor_tensor(dxh[:], fdx[:], bdx[:], A.add)   # E - W
    e_diff.tensor_tensor(syh[:], fdy[:], bdy[:], A.add)   # N - S = -dyh

    e_adv.tensor_tensor(dxh[:], ut[:], dxh[:], A.mult)    # u*(E-W)
    e_adv.tensor_tensor(vt[:], vt[:], syh[:], A.mult)     # v*(N-S)
    e_adv.tensor_tensor(dxh[:], dxh[:], vt[:], A.subtract)  # advh

    # em2 = 0.5*(b != 1)*(b != 2);  le starts as em2*advh (folds the 0.5
    # from dx = 0.5*(E-W))
    v.tensor_scalar(em2[:], bi, 1.0, 0.5, A.not_equal, A.mult)
    v.scalar_tensor_tensor(em2[:], bi, 2.0, em2[:], A.not_equal, A.mult)
    v.scalar_tensor_tensor(eqt[:], em2[:], 2.0, bi, A.mult, A.mult)  # b*em
    v.tensor_tensor(dxh[:], em2[:], dxh[:], A.mult)   # adv*em

    # D accumulation: 4 direction mask-sums, chained per direction
    dirs = [
        (4.0, 8.0, 11.0, fdx),
        (5.0, 8.0, 9.0, fdy),
        (6.0, 9.0, 10.0, bdx),
        (7.0, 10.0, 11.0, bdy),
    ]
    act_masks = os.environ.get("ACT_MASKS", "1") == "1"
    for i, (k1, k2, k3, dt_) in enumerate(dirs):
        cm = em2  # em2 is dead after the adv*em mult above
        if act_masks:
            # e_k1 = relu(1 - |b - k1|) on the scalar engine (exact for
            # integer-valued b), freeing DVE cycles
            sc.activation(cm[:], bi, ACTF.Abs, bias=-k1)
            sc.activation(cm[:], cm[:], ACTF.Relu, bias=1.0, scale=-1.0)
        else:
            v.tensor_single_scalar(cm[:], bi, k1, A.is_equal)
        v.scalar_tensor_tensor(cm[:], bi, k2, cm[:], A.is_equal, A.add)
        v.scalar_tensor_tensor(cm[:], bi, k3, cm[:], A.is_equal, A.add)
        if i == 0:
            e_prod.tensor_tensor(syh[:], dt_[:], cm[:], A.mult)   # D = fdx*c0
        else:
            e_prod.tensor_tensor(dt_[:], dt_[:], cm[:], A.mult)
            e_dadd.tensor_tensor(syh[:], syh[:], dt_[:], A.add)   # D += ...

    v.tensor_tensor(dxh[:], dxh[:], syh[:], A.add)    # le = adv*em + D

    sc.activation(fdx[:], syh[:], ACTF.Abs)           # |D|
    v.tensor_scalar(fdy[:], bi, 3.5, float(FLUX), A.is_gt, A.mult)
    v.tensor_scalar(bdx[:], bi, 7.5, float(DELTA), A.is_gt, A.mult)
    v.tensor_tensor(fdy[:], fdy[:], bdx[:], A.add)    # F_amt
    v.tensor_tensor(fdx[:], fdx[:], fdy[:], A.subtract)  # lf = |D| - F_amt

    sc.activation(bdy[:], dxh[:], ACTF.Square, accum_out=acc_e[:])
    sc.activation(bdx[:], fdx[:], ACTF.Square, accum_out=acc_f[:])

    nc.sync.dma_start(out=hbc_h.ap()[sl, :], in_=hbc[:, 1 : G + 1, 1 : WCH + 1])
    nc.sync.dma_start(out=eq_h.ap()[sl, :], in_=eqt[:])
    nc.sync.dma_start(out=acc_h.ap()[2 * blk : 2 * blk + 1, :], in_=acc_e[:])
    nc.sync.dma_start(out=acc_h.ap()[2 * blk + 1 : 2 * blk + 2, :], in_=acc_f[:])


NPATCH = NBLK * 128          # 512 patches per core
FHALO = (G + 2) * (WCH + 2)  # 2580 elems per halo patch
FINT = G * WCH               # 2048 elems per interior patch


def _build_nc():
    nc = bass.Bass("TRN2", target_bir_lowering=False, debug=False)
    heat_h = nc.dram_tensor("heat_in", [NPATCH, FHALO], F32, kind="ExternalInput")
    b_h = nc.dram_tensor("b_in", [NPATCH, FHALO], U8, kind="ExternalInput")
    u_h = nc.dram_tensor("u_in", [NPATCH, FINT], BF16, kind="ExternalInput")
    v_h = nc.dram_tensor("v_in", [NPATCH, FINT], BF16, kind="ExternalInput")
    hbc_h = nc.dram_tensor("hbc_out", [NPATCH, FINT], F32, kind="ExternalOutput")
    eq_h = nc.dram_tensor("eq_out", [NPATCH, FINT], BF16, kind="ExternalOutput")
    acc_h = nc.dram_tensor("acc_out", [2 * NBLK, 128], F32, kind="ExternalOutput")
    handles = (heat_h, b_h, u_h, v_h, hbc_h, eq_h, acc_h)

    # bias constants for the scalar-engine mask trick (activation bias must
    # come from the const pool)
    for val in (-4.0, -5.0, -6.0, -7.0):
        t = nc.alloc_sbuf_tensor(f"const-float32-{val}", [128, 1], F32)
        nc.gpsimd.memset(t.ap(), val)
        nc.const_aps.aps[(F32, val)] = t.ap()
    nc.all_engine_barrier()

    with tile.TileContext(nc) as tc:
        with (
            tc.tile_pool(name="pio", bufs=2) as pio,
            tc.tile_pool(name="pscr", bufs=1) as pscr,
            tc.tile_pool(name="pacc", bufs=1) as pacc,
        ):
            for blk in range(NBLK):
                _emit_block(nc, pio, pscr, pacc, handles, blk)

    _split_excess_waits(nc)
    return nc


# ------------------------------------------------------------------ runner
def _make_runner(nc):
    import jax
    from jax.sharding import Mesh, PartitionSpec

    try:
        from jax.experimental.shard_map import shard_map
    except ImportError:  # newer jax
        from jax.shard_map import shard_map

    from concourse import bass2jax

    bass2jax.install_neuronx_cc_hook()

    partition_name = (
        nc.partition_id_tensor.name if nc.partition_id_tensor else None
    )
    in_names, out_names, out_avals = [], [], []
    for alloc in nc.m.functions[0].allocations:
        if not isinstance(alloc, mybir.MemoryLocationSet):
            continue
        name = alloc.memorylocations[0].name
        if alloc.kind == "ExternalInput":
            if name != partition_name:
                in_names.append(name)
        elif alloc.kind == "ExternalOutput":
            out_names.append(name)
            out_avals.append(
                jax.core.ShapedArray(
                    tuple(alloc.tensor_shape), mybir.dt.np(alloc.dtype)
                )
            )
    n_params = len(in_names)
    bind_names = list(in_names) + list(out_names)
    if partition_name is not None:
        bind_names.append(partition_name)
    bind_names = tuple(bind_names)

    def _body(*args):
        operands = list(args)
        if partition_name is not None:
            operands.append(bass2jax.partition_id_tensor())
        outs = bass2jax._bass_exec_p.bind(
            *operands,
            out_avals=tuple(out_avals),
            in_names=bind_names,
            out_names=tuple(out_names),
            lowering_input_output_aliases=(),
            sim_require_finite=True,
            sim_require_nnan=True,
            nc=nc,
        )
        return tuple(outs)

    devices = jax.devices()[:NCORES]
    mesh = Mesh(np.asarray(devices), ("core",))
    nops = n_params + len(out_names)
    fn = jax.jit(
        shard_map(
            _body,
            mesh=mesh,
            in_specs=(PartitionSpec("core"),) * nops,
            out_specs=(PartitionSpec("core"),) * len(out_names),
            check_rep=False,
        ),
        keep_unused=True,
    )

    # output placeholder buffers (contents unused; every output element is
    # written by the kernel) - allocate once, reuse across calls
    zero_outs = [
        jax.device_put(
            np.zeros((NCORES * av.shape[0], *av.shape[1:]), av.dtype),
            jax.sharding.NamedSharding(mesh, PartitionSpec("core")),
        )
        for av in out_avals
    ]
    return fn, in_names, out_names, zero_outs, mesh


@functools.lru_cache(maxsize=1)
def _get_runner():
    return _make_runner(_build_nc())


@functools.lru_cache(maxsize=1)
def _get_trivial_runner():
    """Minimal kernel through the same dispatch path, for overhead
    baselining in timing."""
    import jax
    from jax.sharding import NamedSharding, PartitionSpec

    nc = bass.Bass("TRN2", target_bir_lowering=False, debug=False)
    x = nc.dram_tensor("x", [128, 16], F32, kind="ExternalInput")
    y = nc.dram_tensor("y", [128, 16], F32, kind="ExternalOutput")
    with tile.TileContext(nc) as tc:
        with tc.tile_pool(name="p", bufs=1) as pool:
            t = pool.tile([128, 16], F32)
            nc.sync.dma_start(out=t[:], in_=x.ap())
            nc.sync.dma_start(out=y.ap(), in_=t[:])
    _split_excess_waits(nc)
    fn, in_names, out_names, zero_outs, mesh = _make_runner(nc)
    sh = NamedSharding(mesh, PartitionSpec("core"))
    ins = [jax.device_put(np.zeros((NCORES * 128, 16), np.float32), sh)]
    return fn, ins, zero_outs


def _prep_inputs(layout, heat, flow):
    """Host-side shard prep: boundary edits, reflect padding, overlapping
    row slices per core. Returns dict name -> concatenated [8*rows, cols]."""
    heat2 = np.asarray(heat, dtype=np.float32).reshape(H, W)
    u = np.ascontiguousarray(np.asarray(flow, dtype=np.float32)[0, 0])
    v = np.ascontiguousarray(np.asarray(flow, dtype=np.float32)[0, 1])
    b = np.array(np.asarray(layout, dtype=np.float32)[0, 1])  # copy

    # boundary edits (order matters; mirrors the reference)
    b[1, 1:] = 0.0
    b[-2, 1:] = 0.0
    b[:, 1] = 0.0
    b[:, -1] = 3.0
    b[0, :] = 3.0
    b[-1, :] = 3.0

    def pad_reflect(x):
        p = np.empty((H + 2, W + 2), np.float32)
        p[1:-1, 1:-1] = x
        p[0, 1:-1] = x[1]
        p[-1, 1:-1] = x[-2]
        p[:, 0] = p[:, 2]
        p[:, -1] = p[:, -3]
        return p

    hp = pad_reflect(heat2)
    bp = pad_reflect(b)

    def tile_halo(p):
        # [H+2, W+2] -> patch-major [8*512, 2580] (halo rows/cols duplicated)
        wins = np.lib.stride_tricks.sliding_window_view(p, (G + 2, WCH + 2))
        t = wins[::G, ::WCH]  # [256 bands, 16 chunks, 10, 258]
        return np.ascontiguousarray(t).reshape(NCORES * NPATCH, FHALO)

    def tile_int(x):
        # [H, W] -> patch-major [8*512, 2048]
        t = x.reshape(H // G, G, CCH, WCH).transpose(0, 2, 1, 3)
        return np.ascontiguousarray(t).reshape(NCORES * NPATCH, FINT)

    bf16 = mybir.dt.np(BF16)
    return (
        {
            "heat_in": tile_halo(hp),
            "b_in": tile_halo(bp).astype(np.uint8),
            "u_in": tile_int(u).astype(bf16),
            "v_in": tile_int(v).astype(bf16),
        },
        b,
    )


def _untile_int(x):
    # patch-major [8*512, 2048] -> [H, W]
    t = x.reshape(H // G, CCH, G, WCH).transpose(0, 2, 1, 3)
    return np.ascontiguousarray(t).reshape(H, W)


def run_on_device(concat_ins):
    """Run the SPMD kernel on 8 cores. Returns dict name -> concatenated
    output array (host numpy)."""
    import jax

    fn, in_names, out_names, zero_outs, mesh = _get_runner()
    from jax.sharding import NamedSharding, PartitionSpec

    sh = NamedSharding(mesh, PartitionSpec("core"))
    dev_ins = [jax.device_put(concat_ins[n], sh) for n in in_names]
    outs = fn(*dev_ins, *zero_outs)
    return {n: np.asarray(o) for n, o in zip(out_names, outs)}


def kernel(layout, heat, flow):
    concat_ins, _ = _prep_inputs(layout, heat, flow)
    res = run_on_device(concat_ins)

    hbc_full = _untile_int(res["hbc_out"]).reshape(1, 1, H, W)
    eq_full = _untile_int(res["eq_out"].astype(np.float32))
    acc = res["acc_out"].reshape(NCORES, 2 * NBLK, 128)
    sum_e = acc[:, 0::2, :].astype(np.float64).sum()
    sum_f = acc[:, 1::2, :].astype(np.float64).sum()
    mse = np.float32(np.float32(sum_e / (H * W)) + np.float32(sum_f / (H * W)))
    return mse, hbc_full, eq_full


# revision 34
# speedup vs baseline: 1.7099x; 1.7099x over previous
"""Trainium2 Bass kernel for nn_Energy_layer (stencil energy/flux losses).

Contract: kernel(layout, heat, flow) takes FULL inputs, returns the FULL
output tuple (mse_energy + mse_flux, heat_bc, eq_mask) matching reference().

Strategy: rows are sharded across 8 NeuronCores (256 rows each, 1-row halo
supplied host-side via overlapping slices of the reflect-padded arrays).
On each core the image is processed as 512 patches of 8x256 interior with a
1-ring halo; each SBUF partition holds one patch, so every 3x3-stencil shift
is a free-dimension shift. Masking/loss algebra uses the identity

    loss_energy = adv*em + D,  loss_flux = |D| - F_amt(b),
    D = fdx*[b in {4,8,11}] + fdy*[b in {5,8,9}] + bdx*[b in {6,9,10}]
        + bdy*[b in {7,10,11}]

which collapses the 10 reference masks into 4 mask-sums shared by both
losses. Per-partition partial sums of loss^2 are accumulated on the scalar
engine (Square activation with accum_out) and reduced on host.
"""

import functools

import numpy as np

import concourse.bass as bass
import concourse.mybir as mybir
import concourse.tile as tile
from concourse.ap import AP
from concourse.vector_clock import ScopedClock

A = mybir.AluOpType
F32 = mybir.dt.float32
BF16 = mybir.dt.bfloat16
U8 = mybir.dt.uint8
ACTF = mybir.ActivationFunctionType

# ---------------------------------------------------------------- geometry
H, W = 2048, 4096
NCORES = 8
R = H // NCORES            # 256 rows per core
G = 8                      # patch interior rows
WCH = 256                  # patch interior cols
CCH = W // WCH             # 16 col chunks
BPB = 8                    # bands per block (8 bands x 16 chunks = 128 patches)
NBLK = (R // G) // BPB     # 4 blocks per core
WIN = W + 2                # 4098 input cols (with reflect halo)
RIN = R + 2                # 258 input rows (with halo)

FLUX = np.float32(300.0 * (6.0 / 4096))
SQRT2 = np.float32(1.41421)
SQ2F = np.float32(SQRT2 * FLUX)
DELTA = np.float32(SQ2F - FLUX)   # exact: F + DELTA == fl(SQRT2*F)

_WAIT_CAP = 1


# ------------------------------------------------- walrus compat workarounds
def _patched_drain_and_barrier(self, tick_clock, wait_clock):
    # This walrus build rejects instructions carrying more than one sync
    # wait; split the tile tail-drain's waits across multiple drains.
    nc = self.nc
    drain_inst = nc.sync.drain()
    wait_clock.add_sem_waits(
        drain_inst.ins, ScopedClock({None: tick_clock.global_clock})
    )
    si = drain_inst.ins.sync_info
    waits = list(si.on_wait or [])
    if len(waits) > _WAIT_CAP:
        si.on_wait = waits[:_WAIT_CAP]
        rest = waits[_WAIT_CAP:]
        while rest:
            d2 = nc.sync.drain()
            si2 = d2.ins.sync_info
            if si2 is None:
                d2.ins.sync_info = mybir.SyncInfo(
                    on_update=[], on_wait=rest[:_WAIT_CAP]
                )
            else:
                si2.on_wait = rest[:_WAIT_CAP]
            rest = rest[_WAIT_CAP:]
    nc.all_engine_barrier()
    assert self.sems is not None
    popped = nc._tile_sem_poison_stack.pop()
    assert popped is self._sem_poison
    nc.clear_and_free_semaphores(list(self.sems.allocated().values()))
    nc.all_engine_barrier()


tile.TileContext._drain_and_barrier = _patched_drain_and_barrier


def _split_excess_waits(nc, cap=_WAIT_CAP):
    # Move excess sem waits onto NoOps inserted before the instruction on
    # the same engine (program order preserves the wait semantics).
    for fn in nc.m.functions:
        for blk in fn.blocks:
            out = []
            changed = False
            for inst in blk.instructions:
                si = inst.sync_info
                waits = list(si.on_wait) if si is not None and si.on_wait else []
                if len(waits) > cap:
                    excess = waits[:-cap]
                    si.on_wait = waits[-cap:]
                    for j in range(0, len(excess), cap):
                        nop = mybir.InstNoOp(
                            name=f"{inst.name}-wsplit{j}", ins=[], outs=[]
                        )
                        nop.engine = inst.engine
                        nop.sync_info = mybir.SyncInfo(
                            on_update=[], on_wait=excess[j : j + cap]
                        )
                        out.append(nop)
                    changed = True
                out.append(inst)
            if changed:
                blk.instructions = out


# ------------------------------------------------------------ kernel build
def _emit_block(nc, pio, pscr, pacc, handles, blk):
    heat_h, b_h, u_h, v_h, hbc_h, eq_h, acc_h = handles

    ht = pio.tile([128, G + 2, WCH + 2], F32, tag="heat")
    bt = pio.tile([128, G + 2, WCH + 2], F32, tag="b")
    bbf = pio.tile([128, G, WCH], BF16, tag="bbf")
    ut = pio.tile([128, G, WCH], BF16, tag="u")
    vt = pio.tile([128, G, WCH], BF16, tag="v")
    hbc = pio.tile([128, G + 2, WCH + 2], F32, tag="hbc")
    eqt = pio.tile([128, G, WCH], BF16, tag="eq")

    # bf16 copies of hbc; b0 plain (E/W views 4B-aligned), b1 shifted left
    # by one col (C/N/S views 4B-aligned) so DVE 2x mode engages everywhere
    hb0 = pio.tile([128, G + 2, WCH + 2], BF16, tag="hb0")
    hb1 = pio.tile([128, G + 2, WCH + 2], BF16, tag="hb1")

    fdx = pscr.tile([128, G, WCH], BF16, tag="fdx")
    fdy = pscr.tile([128, G, WCH], BF16, tag="fdy")
    bdx = pscr.tile([128, G, WCH], BF16, tag="bdx")
    bdy = pscr.tile([128, G, WCH], BF16, tag="bdy")
    dxh = pscr.tile([128, G, WCH], BF16, tag="dxh")
    syh = pscr.tile([128, G, WCH], BF16, tag="syh")
    em2 = pscr.tile([128, G, WCH], BF16, tag="em2")

    acc_e = pacc.tile([128, 1], F32, tag=f"acce{blk}")
    acc_f = pacc.tile([128, 1], F32, tag=f"accf{blk}")

    sl = slice(blk * 128, (blk + 1) * 128)
    nc.sync.dma_start(out=ht[:], in_=heat_h.ap()[sl, :])
    nc.gpsimd.dma_start(out=bt[:], in_=b_h.ap()[sl, :])  # u8 -> f32 cast
    # u8 -> bf16 cast load of the interior of b (strided in DRAM)
    b_int_src = AP(
        b_h,
        blk * 128 * FHALO + (WCH + 2) + 1,
        [[FHALO, 128], [WCH + 2, G], [1, WCH]],
    )
    nc.gpsimd.dma_start(out=bbf[:], in_=b_int_src)
    nc.sync.dma_start(out=ut[:], in_=u_h.ap()[sl, :])
    nc.sync.dma_start(out=vt[:], in_=v_h.ap()[sl, :])

    import os

    # Engine split: keeping ALL elementwise compute on the vector engine
    # measured fastest (GPSIMD tensor_tensor is ~2x slower per op and is
    # already busy generating SWDGE descriptors for the cast-DMAs; the
    # scalar engine only handles converts/abs/square-accumulate).
    gp_ops = set(os.environ.get("GP_OPS", "none").split(","))
    gp = nc.gpsimd
    v = nc.vector
    sc = nc.scalar
    e_diff = gp if "diffs" in gp_ops else v
    e_prod = gp if "prods" in gp_ops else v
    e_dadd = gp if "dadds" in gp_ops else v
    e_adv = gp if "adv" in gp_ops else v

    # hbc = heat * (b != 1)  (full halo tile, f32 - this is an output)
    v.scalar_tensor_tensor(hbc[:], bt[:], 1.0, ht[:], A.not_equal, A.mult)
    # bf16 copies for the stencil math
    sc.copy(hb0[:], hbc[:])
    sc.copy(hb1[:, :, 0 : WCH + 1], hbc[:, :, 1 : WCH + 2])

    # interior views (C at halo col j; even element offsets everywhere)
    C = hb1[:, 1 : G + 1, 0:WCH]
    E = hb0[:, 1 : G + 1, 2 : WCH + 2]
    Wv = hb0[:, 1 : G + 1, 0:WCH]
    Nv = hb1[:, 0:G, 0:WCH]
    Sv = hb1[:, 2 : G + 2, 0:WCH]
    bi = bbf[:]

    e_diff.tensor_tensor(fdx[:], E, C, A.subtract)
    e_diff.tensor_tensor(fdy[:], Nv, C, A.subtract)
    e_diff.tensor_tensor(bdx[:], C, Wv, A.subtract)
    e_diff.tensor_tensor(bdy[:], C, Sv, A.subtract)
    e_diff.tensor_tensor(dxh[:], fdx[:], bdx[:], A.add)   # E - W
    e_diff.tensor_tensor(syh[:], fdy[:], bdy[:], A.add)   # N - S = -dyh

    e_adv.tensor_tensor(dxh[:], ut[:], dxh[:], A.mult)    # u*(E-W)
    e_adv.tensor_tensor(vt[:], vt[:], syh[:], A.mult)     # v*(N-S)
    e_adv.tensor_tensor(dxh[:], dxh[:], vt[:], A.subtract)  # advh

    # em2 = 0.5*(b != 1)*(b != 2);  le starts as em2*advh (folds the 0.5
    # from dx = 0.5*(E-W))
    v.tensor_scalar(em2[:], bi, 1.0, 0.5, A.not_equal, A.mult)
    v.scalar_tensor_tensor(em2[:], bi, 2.0, em2[:], A.not_equal, A.mult)
    v.scalar_tensor_tensor(eqt[:], em2[:], 2.0, bi, A.mult, A.mult)  # b*em
    v.tensor_tensor(dxh[:], em2[:], dxh[:], A.mult)   # adv*em

    # D accumulation: 4 direction mask-sums, chained per direction
    dirs = [
        (4.0, 8.0, 11.0, fdx),
        (5.0, 8.0, 9.0, fdy),
        (6.0, 9.0, 10.0, bdx),
        (7.0, 10.0, 11.0, bdy),
    ]
    for i, (k1, k2, k3, dt_) in enumerate(dirs):
        cm = em2  # em2 is dead after the adv*em mult above
        v.tensor_single_scalar(cm[:], bi, k1, A.is_equal)
        v.scalar_tensor_tensor(cm[:], bi, k2, cm[:], A.is_equal, A.add)
        v.scalar_tensor_tensor(cm[:], bi, k3, cm[:], A.is_equal, A.add)
        if i == 0:
            e_prod.tensor_tensor(syh[:], dt_[:], cm[:], A.mult)   # D = fdx*c0
        else:
            e_prod.tensor_tensor(dt_[:], dt_[:], cm[:], A.mult)
            e_dadd.tensor_tensor(syh[:], syh[:], dt_[:], A.add)   # D += ...

    v.tensor_tensor(dxh[:], dxh[:], syh[:], A.add)    # le = adv*em + D

    sc.activation(fdx[:], syh[:], ACTF.Abs)           # |D|
    v.tensor_scalar(fdy[:], bi, 3.5, float(FLUX), A.is_gt, A.mult)
    v.tensor_scalar(bdx[:], bi, 7.5, float(DELTA), A.is_gt, A.mult)
    v.tensor_tensor(fdy[:], fdy[:], bdx[:], A.add)    # F_amt
    v.tensor_tensor(fdx[:], fdx[:], fdy[:], A.subtract)  # lf = |D| - F_amt

    sc.activation(bdy[:], dxh[:], ACTF.Square, accum_out=acc_e[:])
    sc.activation(bdx[:], fdx[:], ACTF.Square, accum_out=acc_f[:])

    nc.sync.dma_start(out=hbc_h.ap()[sl, :], in_=hbc[:, 1 : G + 1, 1 : WCH + 1])
    nc.sync.dma_start(out=eq_h.ap()[sl, :], in_=eqt[:])
    nc.sync.dma_start(out=acc_h.ap()[2 * blk : 2 * blk + 1, :], in_=acc_e[:])
    nc.sync.dma_start(out=acc_h.ap()[2 * blk + 1 : 2 * blk + 2, :], in_=acc_f[:])


NPATCH = NBLK * 128          # 512 patches per core
FHALO = (G + 2) * (WCH + 2)  # 2580 elems per halo patch
FINT = G * WCH               # 2048 elems per interior patch


def _build_nc():
    nc = bass.Bass("TRN2", target_bir_lowering=False, debug=False)
    heat_h = nc.dram_tensor("heat_in", [NPATCH, FHALO], F32, kind="ExternalInput")
    b_h = nc.dram_tensor("b_in", [NPATCH, FHALO], U8, kind="ExternalInput")
    u_h = nc.dram_tensor("u_in", [NPATCH, FINT], BF16, kind="ExternalInput")
    v_h = nc.dram_tensor("v_in", [NPATCH, FINT], BF16, kind="ExternalInput")
    hbc_h = nc.dram_tensor("hbc_out", [NPATCH, FINT], F32, kind="ExternalOutput")
    eq_h = nc.dram_tensor("eq_out", [NPATCH, FINT], BF16, kind="ExternalOutput")
    acc_h = nc.dram_tensor("acc_out", [2 * NBLK, 128], F32, kind="ExternalOutput")
    handles = (heat_h, b_h, u_h, v_h, hbc_h, eq_h, acc_h)

    with tile.TileContext(nc) as tc:
        with (
            tc.tile_pool(name="pio", bufs=2) as pio,
            tc.tile_pool(name="pscr", bufs=1) as pscr,
            tc.tile_pool(name="pacc", bufs=1) as pacc,
        ):
            for blk in range(NBLK):
                _emit_block(nc, pio, pscr, pacc, handles, blk)

    _split_excess_waits(nc)
    return nc


# ------------------------------------------------------------------ runner
def _make_runner(nc):
    import jax
    from jax.sharding import Mesh, PartitionSpec

    try:
        from jax.experimental.shard_map import shard_map
    except ImportError:  # newer jax
        from jax.shard_map import shard_map

    from concourse import bass2jax

    bass2jax.install_neuronx_cc_hook()

    partition_name = (
        nc.partition_id_tensor.name if nc.partition_id_tensor else None
    )
    in_names, out_names, out_avals = [], [], []
    for alloc in nc.m.functions[0].allocations:
        if not isinstance(alloc, mybir.MemoryLocationSet):
            continue
        name = alloc.memorylocations[0].name
        if alloc.kind == "ExternalInput":
            if name != partition_name:
                in_names.append(name)
        elif alloc.kind == "ExternalOutput":
            out_names.append(name)
            out_avals.append(
                jax.core.ShapedArray(
                    tuple(alloc.tensor_shape), mybir.dt.np(alloc.dtype)
                )
            )
    n_params = len(in_names)
    bind_names = list(in_names) + list(out_names)
    if partition_name is not None:
        bind_names.append(partition_name)
    bind_names = tuple(bind_names)

    def _body(*args):
        operands = list(args)
        if partition_name is not None:
            operands.append(bass2jax.partition_id_tensor())
        outs = bass2jax._bass_exec_p.bind(
            *operands,
            out_avals=tuple(out_avals),
            in_names=bind_names,
            out_names=tuple(out_names),
            lowering_input_output_aliases=(),
            sim_require_finite=True,
            sim_require_nnan=True,
            nc=nc,
        )
        return tuple(outs)

    devices = jax.devices()[:NCORES]
    mesh = Mesh(np.asarray(devices), ("core",))
    nops = n_params + len(out_names)
    fn = jax.jit(
        shard_map(
            _body,
            mesh=mesh,
            in_specs=(PartitionSpec("core"),) * nops,
            out_specs=(PartitionSpec("core"),) * len(out_names),
            check_rep=False,
        ),
        keep_unused=True,
    )

    # output placeholder buffers (contents unused; every output element is
    # written by the kernel) - allocate once, reuse across calls
    zero_outs = [
        jax.device_put(
            np.zeros((NCORES * av.shape[0], *av.shape[1:]), av.dtype),
            jax.sharding.NamedSharding(mesh, PartitionSpec("core")),
        )
        for av in out_avals
    ]
    return fn, in_names, out_names, zero_outs, mesh


@functools.lru_cache(maxsize=1)
def _get_runner():
    return _make_runner(_build_nc())


@functools.lru_cache(maxsize=1)
def _get_trivial_runner():
    """Minimal kernel through the same dispatch path, for overhead
    baselining in timing."""
    import jax
    from jax.sharding import NamedSharding, PartitionSpec

    nc = bass.Bass("TRN2", target_bir_lowering=False, debug=False)
    x = nc.dram_tensor("x", [128, 16], F32, kind="ExternalInput")
    y = nc.dram_tensor("y", [128, 16], F32, kind="ExternalOutput")
    with tile.TileContext(nc) as tc:
        with tc.tile_pool(name="p", bufs=1) as pool:
            t = pool.tile([128, 16], F32)
            nc.sync.dma_start(out=t[:], in_=x.ap())
            nc.sync.dma_start(out=y.ap(), in_=t[:])
    _split_excess_waits(nc)
    fn, in_names, out_names, zero_outs, mesh = _make_runner(nc)
    sh = NamedSharding(mesh, PartitionSpec("core"))
    ins = [jax.device_put(np.zeros((NCORES * 128, 16), np.float32), sh)]
    return fn, ins, zero_outs


def _prep_inputs(layout, heat, flow):
    """Host-side shard prep: boundary edits, reflect padding, overlapping
    row slices per core. Returns dict name -> concatenated [8*rows, cols]."""
    heat2 = np.asarray(heat, dtype=np.float32).reshape(H, W)
    u = np.ascontiguousarray(np.asarray(flow, dtype=np.float32)[0, 0])
    v = np.ascontiguousarray(np.asarray(flow, dtype=np.float32)[0, 1])
    b = np.array(np.asarray(layout, dtype=np.float32)[0, 1])  # copy

    # boundary edits (order matters; mirrors the reference)
    b[1, 1:] = 0.0
    b[-2, 1:] = 0.0
    b[:, 1] = 0.0
    b[:, -1] = 3.0
    b[0, :] = 3.0
    b[-1, :] = 3.0

    def pad_reflect(x):
        p = np.empty((H + 2, W + 2), np.float32)
        p[1:-1, 1:-1] = x
        p[0, 1:-1] = x[1]
        p[-1, 1:-1] = x[-2]
        p[:, 0] = p[:, 2]
        p[:, -1] = p[:, -3]
        return p

    hp = pad_reflect(heat2)
    bp = pad_reflect(b)

    def tile_halo(p):
        # [H+2, W+2] -> patch-major [8*512, 2580] (halo rows/cols duplicated)
        wins = np.lib.stride_tricks.sliding_window_view(p, (G + 2, WCH + 2))
        t = wins[::G, ::WCH]  # [256 bands, 16 chunks, 10, 258]
        return np.ascontiguousarray(t).reshape(NCORES * NPATCH, FHALO)

    def tile_int(x):
        # [H, W] -> patch-major [8*512, 2048]
        t = x.reshape(H // G, G, CCH, WCH).transpose(0, 2, 1, 3)
        return np.ascontiguousarray(t).reshape(NCORES * NPATCH, FINT)

    bf16 = mybir.dt.np(BF16)
    return (
        {
            "heat_in": tile_halo(hp),
            "b_in": tile_halo(bp).astype(np.uint8),
            "u_in": tile_int(u).astype(bf16),
            "v_in": tile_int(v).astype(bf16),
        },
        b,
    )


def _untile_int(x):
    # patch-major [8*512, 2048] -> [H, W]
    t = x.reshape(H // G, CCH, G, WCH).transpose(0, 2, 1, 3)
    return np.ascontiguousarray(t).reshape(H, W)


def run_on_device(concat_ins):
    """Run the SPMD kernel on 8 cores. Returns dict name -> concatenated
    output array (host numpy)."""
    import jax

    fn, in_names, out_names, zero_outs, mesh = _get_runner()
    from jax.sharding import NamedSharding, PartitionSpec

    sh = NamedSharding(mesh, PartitionSpec("core"))
    dev_ins = [jax.device_put(concat_ins[n], sh) for n in in_names]
    outs = fn(*dev_ins, *zero_outs)
    return {n: np.asarray(o) for n, o in zip(out_names, outs)}


def kernel(layout, heat, flow):
    concat_ins, _ = _prep_inputs(layout, heat, flow)
    res = run_on_device(concat_ins)

    hbc_full = _untile_int(res["hbc_out"]).reshape(1, 1, H, W)
    eq_full = _untile_int(res["eq_out"].astype(np.float32))
    acc = res["acc_out"].reshape(NCORES, 2 * NBLK, 128)
    sum_e = acc[:, 0::2, :].astype(np.float64).sum()
    sum_f = acc[:, 1::2, :].astype(np.float64).sum()
    mse = np.float32(np.float32(sum_e / (H * W)) + np.float32(sum_f / (H * W)))
    return mse, hbc_full, eq_full
